# revision 19
# baseline (speedup 1.0000x reference)
"""Trainium2 Bass kernel for nn_MultiHeadAttention (B=2, S=2048, E=1024,
H=16, D=64) on 8 NeuronCores.

Sharding: core c -> (batch b = c//4, head-group g = c%4). Each core
computes Q/K/V projections for its batch restricted to its 4 heads
(column-parallel Wq/Wk/Wv), full attention for those heads, and a
row-parallel partial fc_out. The host sums the 4 partial outputs per
batch in fp32 and adds bo once.

Engine budget per core (all bf16 on the PE, f32 psum):
  ACT: 128 exps of [128, 1024] (~1.09us each)        ~139us
  PE:  scores 27us (K=64 row-tile pairs, concurrent)
       AV     55us (K=128, M=65 with ones-col denominator)
       Q/K    27us, V 14us (bias adds moved to DVE drains)
       fc     14us (row-tile paired K=64 streams, 2 psum tiles,
                    DVE-add drain)                    ~137us
The kernel is a single exp-paced software pipeline:
  - minimal lead-in: only K^T (both j) and Q^T chunk 0 run before the
    first exp; the DMA order (Wk, xk, Wq, xq, Wv, xv, Wo) gates it.
  - V projection, remaining Q^T chunks and fc are drip-fed into the
    per-iteration PE slack (~0.4us) as fine-grained filler items.
  - AV matmuls trail the exp stream through a backlog deque; the first
    pair uses a deep backlog (DEPTH_FIRST) to bridge the xv DMA.
  - softmax tail per pair: po -> raw (DVE), reciprocal_approx_fast on
    the denominator row (partition 0), gpsimd broadcast, one DVE mul
    writing the fc-ready ot tile ([128, 512]: head-even rows 0-63,
    head-odd rows 64-127 via cross-base DVE writes).
PSUM (8 banks): scores 2x[128,1024]=4, po 2x[65,512]=2, fc 2x[128,512]=2.
"""

import numpy as np
from collections import deque
from contextlib import ExitStack

import concourse.tile as tile
from concourse import bacc, mybir
from concourse.bass_utils import run_bass_kernel_spmd

F32 = mybir.dt.float32
BF16 = mybir.dt.bfloat16
AF = mybir.ActivationFunctionType

B, S, E, H, D = 2, 2048, 1024, 16, 64
HL = 4            # heads per core
FL = HL * D       # local feature slice (256)
N_CORES = 8

PAIR_FC = False    # row-tile paired fc (needs cross-base DVE writes)
DEPTH_FIRST = 4   # AV backlog for the first pair (bridges xv DMA)
DEPTH = 2         # steady-state AV backlog


def build_nc(S=2048, E=1024):
    T = E // 128       # emb k-tiles (8)
    C = S // 512       # 512-wide seq chunks (4)
    QW = 512
    NQC = S // QW
    NKT = S // 128     # key tiles (16)
    scale = 1.0 / (E ** 0.5)

    nc = bacc.Bacc("TRN2", target_bir_lowering=False, debug=False)

    xqT = nc.dram_tensor("xqT", [E, S], BF16, kind="ExternalInput").ap()
    xkT = nc.dram_tensor("xkT", [E, S], BF16, kind="ExternalInput").ap()
    xvT = nc.dram_tensor("xvT", [E, S], BF16, kind="ExternalInput").ap()
    Wq = nc.dram_tensor("Wq", [128, T * 256], BF16, kind="ExternalInput").ap()
    Wk = nc.dram_tensor("Wk", [128, T * 256], BF16, kind="ExternalInput").ap()
    Wv = nc.dram_tensor("Wv", [128, T * 260], BF16, kind="ExternalInput").ap()
    bqk = nc.dram_tensor("bqk", [128, 4], F32, kind="ExternalInput").ap()
    bvb = nc.dram_tensor("bvb", [128, 260], BF16, kind="ExternalInput").ap()
    WoT = nc.dram_tensor("WoT", [128, 2 * E] if PAIR_FC else [65, 4 * E],
                         BF16, kind="ExternalInput").ap()
    out = nc.dram_tensor("out", [S, E], BF16, kind="ExternalOutput").ap()

    with tile.TileContext(nc) as tc, ExitStack() as ctx:
        const = ctx.enter_context(tc.tile_pool(name="const", bufs=1))
        persist = ctx.enter_context(tc.tile_pool(name="persist", bufs=1))

        wq_sb = const.tile([128, T * 256], BF16)
        wk_sb = const.tile([128, T * 256], BF16)
        wv_sb = const.tile([128, T * 260], BF16)
        wo_sb = const.tile([128, 2 * E] if PAIR_FC else [65, 4 * E], BF16)
        bqk_sb = const.tile([128, 4], F32)
        bvb_sb = const.tile([128, 260], BF16)

        xk_h = [persist.tile([128, 4 * S], BF16, tag=f"xk{i}", name=f"xk{i}") for i in range(2)]
        xq_h = [persist.tile([128, 4 * S], BF16, tag=f"xq{i}", name=f"xq{i}") for i in range(2)]
        xv_h = [persist.tile([128, 4 * S], BF16, tag=f"xv{i}", name=f"xv{i}") for i in range(2)]

        def load_half(x_dram, halves, i):
            src = x_dram.rearrange("(t p) s -> p t s", p=128)
            nc.sync.dma_start(
                halves[i][:].rearrange("p (t s) -> p t s", s=S),
                src[:, 4 * i : 4 * i + 4, :],
            )

        # DMA order gates the pipeline start: first halves of K/Q/V, then
        # second halves, then Wo.
        nc.sync.dma_start(wk_sb[:], Wk)
        nc.sync.dma_start(bqk_sb[:], bqk)
        load_half(xkT, xk_h, 0)
        load_half(xkT, xk_h, 1)
        nc.sync.dma_start(wq_sb[:], Wq)
        load_half(xqT, xq_h, 0)
        load_half(xqT, xq_h, 1)
        nc.sync.dma_start(wv_sb[:], Wv)
        nc.sync.dma_start(bvb_sb[:], bvb)
        load_half(xvT, xv_h, 0)
        load_half(xvT, xv_h, 1)
        nc.sync.dma_start(wo_sb[:], WoT)

        # PE warm-up while the first DMAs land + exp table preload.
        with tc.tile_pool(name="wu", bufs=1) as wu_pool, \
             tc.tile_pool(name="wup", bufs=4, space="PSUM") as wup_pool:
            wu = wu_pool.tile([128, 640], BF16)
            nc.gpsimd.memset(wu[:], 0.0)
            wux = wu_pool.tile([1, 32], F32, name="wux")
            nc.scalar.activation(wux[:], wu[0:1, 0:32], AF.Exp, scale=1.0)
            for i in range(12):
                wp = wup_pool.tile([128, 512], F32, tag="wup", name="wup")
                nc.tensor.matmul(wp[:], wu[:, 0:128], wu[:, 128:640],
                                 start=True, stop=True)

        qt_sb = [persist.tile([128, S], BF16, tag=f"qt{j}", name=f"qt{j}")
                 for j in range(2)]
        kt_sb = [persist.tile([128, S], BF16, tag=f"kt{j}", name=f"kt{j}")
                 for j in range(2)]
        v_sb = persist.tile([128, NKT * 260], BF16, tag="v", name="v_sb")

        qk_pool = [None]

        def xslice(halves, t, lo, w):
            i, tt = t // 4, t % 4
            return halves[i][:, tt * S + lo : tt * S + lo + w]

        def qk_part1(x_h, w_sb, j, lo, w, cell):
            ps = qk_pool[0].tile([128, w], F32, tag="psfA", name="psfA")
            cell.append(ps)
            for t in range(4):
                nc.tensor.matmul(
                    ps[:],
                    w_sb[:, t * 256 + j * 128 : t * 256 + j * 128 + 128],
                    xslice(x_h, t, lo, w),
                    start=(t == 0), stop=False,
                )

        def qk_part2(x_h, w_sb, jcol, dst, j, lo, w, cell):
            ps = cell.pop()
            for t in range(4, T):
                nc.tensor.matmul(
                    ps[:],
                    w_sb[:, t * 256 + j * 128 : t * 256 + j * 128 + 128],
                    xslice(x_h, t, lo, w),
                    start=False, stop=(t == T - 1),
                )
            nc.vector.tensor_scalar_add(
                dst[:, lo : lo + w], ps[:], bqk_sb[:, jcol : jcol + 1])

        def v_part1(kt, cell):
            c, s4 = kt // 4, kt % 4
            ps = qk_pool[0].tile([128, 512], F32, tag="psfB", name="psfB")
            cell.append(ps)
            for t in range(4):
                nc.tensor.matmul(
                    ps[:, 0:260],
                    xslice(xv_h, t, c * 512 + s4 * 128, 128),
                    wv_sb[:, t * 260 : (t + 1) * 260],
                    start=(t == 0), stop=False,
                )

        v_done = [0]

        def v_part2(kt, cell):
            ps = cell.pop()
            for t in range(4, T):
                nc.tensor.matmul(
                    ps[:, 0:260],
                    xslice(xv_h, t, (kt // 4) * 512 + (kt % 4) * 128, 128),
                    wv_sb[:, t * 260 : (t + 1) * 260],
                    start=False, stop=(t == T - 1),
                )
            nc.vector.tensor_tensor(
                v_sb[:, kt * 260 : (kt + 1) * 260],
                ps[:, 0:260], bvb_sb[:], mybir.AluOpType.add)
            v_done[0] = max(v_done[0], kt + 1)

        # ---- lead-in: K^T (both j) + Q^T chunk 0 ----
        filler = deque()
        with tc.tile_pool(name="psA", bufs=2, space="PSUM") as psA:
            qk_pool[0] = psA
            c00 = []
            qk_part1(xk_h, wk_sb, 0, 0, 512, c00)
            qk_part2(xk_h, wk_sb, 2, kt_sb[0], 0, 0, 512, c00)
            q00 = []
            qk_part1(xq_h, wq_sb, 0, 0, 512, q00)
            qk_part2(xq_h, wq_sb, 0, qt_sb[0], 0, 0, 512, q00)

        # deferred projection groups run inside phase B on its psum pool;
        # order matters: K(c,j0) by score kt=4c, then j1's K and Q(c0,j1)
        def add_kq_group(x_h, w_sb, jcol, dst, j, lo):
            cell = []
            filler.append(lambda: qk_part1(x_h, w_sb, j, lo, 512, cell))
            filler.append(lambda: qk_part2(x_h, w_sb, jcol, dst, j, lo, 512,
                                           cell))

        for c in range(1, C):
            add_kq_group(xk_h, wk_sb, 2, kt_sb[0], 0, c * 512)
        add_kq_group(xq_h, wq_sb, 1, qt_sb[1], 1, 0)
        for c in range(C):
            add_kq_group(xk_h, wk_sb, 3, kt_sb[1], 1, c * 512)

        # ---- filler queue: V tiles, Q chunks 1-3, then fc halves ----
        cells_v = {kt: [] for kt in range(16)}
        def add_v(fn):
            fn.is_v = True
            filler.append(fn)

        add_v(lambda: v_part1(0, cells_v[0]))
        add_v(lambda: v_part1(1, cells_v[1]))
        add_v(lambda: v_part2(0, cells_v[0]))
        add_v(lambda: v_part2(1, cells_v[1]))
        for kt in range(2, 4):
            add_v(lambda kt=kt: v_part1(kt, cells_v[kt]))
            add_v(lambda kt=kt: v_part2(kt, cells_v[kt]))

        def add_q_group(c, j):
            cell = []
            filler.append(lambda: qk_part1(xq_h, wq_sb, j, c * 512, 512, cell))
            filler.append(lambda: qk_part2(xq_h, wq_sb, j, qt_sb[j], j,
                                           c * 512, 512, cell))

        for kt in range(4, 8):
            add_v(lambda kt=kt: v_part1(kt, cells_v[kt]))
            add_v(lambda kt=kt: v_part2(kt, cells_v[kt]))
        for j in range(2):
            add_q_group(1, j)
        for kt in range(8, 12):
            add_v(lambda kt=kt: v_part1(kt, cells_v[kt]))
            add_v(lambda kt=kt: v_part2(kt, cells_v[kt]))
        for j in range(2):
            add_q_group(2, j)
        for kt in range(12, 16):
            add_v(lambda kt=kt: v_part1(kt, cells_v[kt]))
            add_v(lambda kt=kt: v_part2(kt, cells_v[kt]))
        for j in range(2):
            add_q_group(3, j)

        fc_queue = deque()

        def pop_work():
            if filler:
                filler.popleft()()
            elif fc_queue:
                fc_queue.popleft()()

        # ---- phase B ----
        with tc.tile_pool(name="pt", bufs=12) as pt_pool, \
             tc.tile_pool(name="fct", bufs=2) as fct, \
             tc.tile_pool(name="raw", bufs=2) as raw_pool, \
             tc.tile_pool(name="rc", bufs=2) as rc_pool, \
             tc.tile_pool(name="bc", bufs=1) as bc_pool, \
             tc.tile_pool(name="ot", bufs=2) as ot_pool, \
             tc.tile_pool(name="os", bufs=2) as os_pool, \
             tc.tile_pool(name="psB_s", bufs=2, space="PSUM") as psB_s, \
             tc.tile_pool(name="psB_o", bufs=1, space="PSUM") as psB_o, \
             tc.tile_pool(name="psB_f", bufs=1, space="PSUM") as psB_f:
            qk_pool[0] = psB_f  # filler projections borrow the fc banks
            fct_pool = [fct]

            def emit_fc_half(qc, ss, e2, ots, cell):
                if e2 == 0:
                    cell.append(os_pool.tile([128, E], BF16, tag="osb",
                                             name="osb"))
                o_sb = cell[0]
                psA_t = psB_f.tile([128, 512], F32, tag="psfA", name="psfA")
                psB_t = psB_f.tile([128, 512], F32, tag="psfB", name="psfB")
                if PAIR_FC:
                    for jj in range(2):
                        nc.tensor.matmul(
                            psA_t[:],
                            ots[jj][0:64, ss * 128 : (ss + 1) * 128],
                            wo_sb[0:64, jj * E + e2 * 512 : jj * E + e2 * 512 + 512],
                            start=(jj == 0), stop=(jj == 1),
                            tile_position=(0, 0),
                        )
                        nc.tensor.matmul(
                            psB_t[:],
                            ots[jj][64:128, ss * 128 : (ss + 1) * 128],
                            wo_sb[64:128, jj * E + e2 * 512 : jj * E + e2 * 512 + 512],
                            start=(jj == 0), stop=(jj == 1),
                            tile_position=(64, 0),
                        )
                    tmp = fct_pool[0].tile([128, 512], F32, tag="fctmp",
                                           name="fctmp")
                    nc.vector.tensor_copy(tmp[:], psA_t[:])
                    nc.vector.tensor_tensor(
                        o_sb[:, e2 * 512 : (e2 + 1) * 512],
                        tmp[:], psB_t[:], mybir.AluOpType.add)
                else:
                    for h in range(4):
                        nc.tensor.matmul(
                            psA_t[:],
                            ots[h][:, ss * 128 : (ss + 1) * 128],
                            wo_sb[:, h * E + e2 * 512 : h * E + e2 * 512 + 512],
                            start=(h == 0), stop=(h == 3),
                        )
                    nc.vector.tensor_copy(
                        o_sb[:, e2 * 512 : (e2 + 1) * 512], psA_t[:])
                if e2 == 1:
                    nc.sync.dma_start(
                        out[qc + ss * 128 : qc + ss * 128 + 128, :],
                        o_sb[:],
                    )

            # AV backlog: callables executed in FIFO order, trailing the
            # exp stream by `target` items.
            av_q = deque()

            def make_tail(po, j, qc, ot_tiles, qw, qlo):
                def tail():
                    raws = []
                    for e in range(2):
                        raw = raw_pool.tile([65, QW], F32, tag=f"raw{e}",
                                            name="raw")[:, 0:qw]
                        nc.vector.tensor_copy(raw[:], po[e][:])
                        raws.append(raw)
                    rcs = []
                    for e in range(2):
                        rc = rc_pool.tile([1, QW], F32, tag=f"rc{e}",
                                          name="rc")[:, 0:qw]
                        dn = 64 if PAIR_FC else 0
                        with nc.allow_low_precision(reason="softmax denom"):
                            nc.vector.reciprocal_approx_fast(
                                rc[:], raws[e][dn : dn + 1, :])
                        rcs.append(rc)
                    bcs = []
                    for e in range(2):
                        bc = bc_pool.tile([65, QW], F32, tag=f"bc{e}",
                                          name="bc")[:, 0:qw]
                        nc.gpsimd.partition_broadcast(bc[:], rcs[e][:])
                        bcs.append(bc)
                    if PAIR_FC:
                        ot = ot_pool.tile([128, QW], BF16, tag=f"ot{j}",
                                          name="ot")
                        nc.vector.tensor_tensor(
                            ot[0:64, :], raws[0][0:64, :], bcs[0][0:64, :],
                            mybir.AluOpType.mult)
                        nc.vector.tensor_tensor(
                            ot[64:128, :], raws[1][0:64, :], bcs[1][0:64, :],
                            mybir.AluOpType.mult)
                        ot_tiles[j] = ot
                    else:
                        for e in range(2):
                            ot = ot_pool.tile([65, QW], BF16,
                                              tag=f"ot{2 * j + e}",
                                              name="ot")[:, 0:qw]
                            nc.vector.tensor_tensor(
                                ot[:], raws[e][:], bcs[e][:],
                                mybir.AluOpType.mult)
                            ot_tiles[2 * j + e] = ot
                    if j == 1:
                        for ss in range(qw // 128):
                            cell = []
                            for e2 in range(2):
                                fc_queue.append(
                                    lambda q=qlo, s=ss, e2=e2,
                                           o=ot_tiles, cell=cell:
                                    emit_fc_half(q, s, e2, o, cell))
                return tail

            score_q = deque()
            CHUNKS = [(0, 512), (512, 512), (1024, 512), (1536, 512)]
            pair_list = [(ci, j) for ci in range(len(CHUNKS))
                         for j in range(2)]

            def emit_score(ci, j, kt):
                qlo, qw = CHUNKS[ci]
                ps_s = psB_s.tile([128, 1024], F32, tag="pss",
                                  name="pss")[:, 0 : 2 * qw]
                for e in range(2):
                    nc.tensor.matmul(
                        ps_s[:, e * qw : (e + 1) * qw],
                        kt_sb[j][64 * e : 64 * e + 64,
                                 kt * 128 : (kt + 1) * 128],
                        qt_sb[j][64 * e : 64 * e + 64, qlo : qlo + qw],
                        start=True, stop=True,
                    )
                return ps_s

            ot_tiles_by_qc = {ci: {} for ci in range(len(CHUNKS))}
            glob_it = [0]
            for pi, (qc, j) in enumerate(pair_list):
                qlo, qw = CHUNKS[qc]
                ot_tiles = ot_tiles_by_qc[qc]
                first_pair = pi == 0
                po = [psB_o.tile([65, QW], F32, tag=f"po{e}",
                                 name=f"po{e}")[:, 0:qw] for e in range(2)]
                pts = {}

                def emit_o(kt, po=po, pts=pts, j=j, qw=qw):
                    for e in range(2):
                        nc.tensor.matmul(
                            po[e][:],
                            v_sb[:, kt * 260 + 65 * (2 * j + e)
                                 : kt * 260 + 65 * (2 * j + e) + 65],
                            pts[kt][:, e * qw : (e + 1) * qw],
                            start=(kt == 0), stop=(kt == NKT - 1),
                        )
                    del pts[kt]

                for kt in range(NKT):
                    if first_pair and kt == 0:
                        score_q.append(emit_score(qc, j, 0))
                    ps_s = score_q.popleft()
                    pt = pt_pool.tile([128, 1024], BF16, tag="pt",
                                      name="pt")[:, 0 : 2 * qw]
                    nc.scalar.activation(pt[:], ps_s[:], AF.Exp, scale=scale)
                    pts[kt] = pt
                    if kt + 1 < NKT:
                        score_q.append(emit_score(qc, j, kt + 1))
                    elif pi + 1 < len(pair_list):
                        nqc, nj = pair_list[pi + 1]
                        score_q.append(emit_score(nqc, nj, 0))
                    av_q.append((kt, lambda kt=kt, f=emit_o: f(kt)))
                    target = DEPTH_FIRST if first_pair else DEPTH
                    pops = 0
                    while len(av_q) > target and pops < 2:
                        nkt, fn = av_q[0]
                        if nkt is not None and nkt >= v_done[0]:
                            break
                        av_q.popleft()
                        fn()
                        pops += 1
                    it = glob_it[0]
                    glob_it[0] = it + 1
                    # no V-proj pops before iter 8: xv DMA is still in
                    # flight and a queued V matmul stalls the whole PE
                    v_ok = it >= 8
                    nf = 2 if (len(av_q) > 5 and filler and v_ok) else 1
                    for _ in range(nf):
                        if filler and (v_ok or not filler[0].__dict__.get
                                       ('is_v', False)):
                            pop_work()
                        elif fc_queue and kt % 2 == 1:
                            pop_work()
                            break
                av_q.append((None, make_tail(po, j, qc, ot_tiles, qw, qlo)))

            # final: drain AV backlog + tails, then qc3's fc.
            while av_q:
                av_q.popleft()[1]()
            for i in range(6):
                ka = psB_s.tile([128, 1024], F32, tag="pss", name="pss")
                nc.tensor.matmul(ka[:, 0:512], kt_sb[0][0:64, 0:128],
                                 qt_sb[0][0:64, 0:512],
                                 start=True, stop=True)
            while fc_queue:
                pop_work()

    nc.compile()
    return nc


_NC_CACHE = [None]


def _get_nc():
    if _NC_CACHE[0] is None:
        _NC_CACHE[0] = build_nc(S=S, E=E)
    return _NC_CACHE[0]


def _pack_w(W):
    """[E, F] -> [128, (E//128)*F], one 128-row k-tile after another."""
    E_, F_ = W.shape
    T_ = E_ // 128
    return np.ascontiguousarray(
        W.reshape(T_, 128, F_).transpose(1, 0, 2).reshape(128, T_ * F_))


def make_in_maps(query, key, value, Wq, bq, Wk, bk, Wv, bv, Wo, bo):
    bf = mybir.dt.np(BF16)
    f32 = np.float32
    query = np.asarray(query, f32)
    key = np.asarray(key, f32)
    value = np.asarray(value, f32)
    Wq, bq = np.asarray(Wq, f32), np.asarray(bq, f32)
    Wk, bk = np.asarray(Wk, f32), np.asarray(bk, f32)
    Wv, bv = np.asarray(Wv, f32), np.asarray(bv, f32)
    Wo, bo = np.asarray(Wo, f32), np.asarray(bo, f32)

    xT = {}
    for b in range(B):
        xT[b] = (
            np.ascontiguousarray(query[b].T).astype(bf),
            np.ascontiguousarray(key[b].T).astype(bf),
            np.ascontiguousarray(value[b].T).astype(bf),
        )

    in_maps = []
    for c in range(N_CORES):
        b, g = c // 4, c % 4
        fs = slice(FL * g, FL * g + FL)
        wq_c = np.ascontiguousarray(Wq[fs, :].T)
        wk_c = np.ascontiguousarray(Wk[fs, :].T)
        wv_c = np.ascontiguousarray(Wv[fs, :].T)
        # V weights with zeroed ones-columns (the DVE bias add supplies
        # the 1.0 denominator column and bv)
        wv_pack = np.zeros((E, HL * 65), f32)
        bv_bcast = np.zeros((128, HL * 65), f32)
        for h in range(HL):
            if PAIR_FC:
                wv_pack[:, 65 * h : 65 * h + 64] = wv_c[:, 64 * h : 64 * h + 64]
                bv_bcast[:, 65 * h : 65 * h + 64] = bv[fs][64 * h : 64 * h + 64][None, :]
                bv_bcast[:, 65 * h + 64] = 1.0
            else:
                bv_bcast[:, 65 * h] = 1.0
                wv_pack[:, 65 * h + 1 : 65 * h + 65] = wv_c[:, 64 * h : 64 * h + 64]
                bv_bcast[:, 65 * h + 1 : 65 * h + 65] = bv[fs][64 * h : 64 * h + 64][None, :]
        # Q/K biases as per-partition columns: bqk[:, 0:2]=bq j=0/1,
        # bqk[:, 2:4]=bk
        bqk = np.zeros((128, 4), f32)
        bqk[:, 0] = bq[fs][0:128]
        bqk[:, 1] = bq[fs][128:256]
        bqk[:, 2] = bk[fs][0:128]
        bqk[:, 3] = bk[fs][128:256]
        if PAIR_FC:
            # [128, 2E]: pair jj -> cols jj*E..; head-even d rows at
            # partitions 0-63, head-odd at 64-127
            wot = np.zeros((128, 2 * E), f32)
            for jj in range(2):
                h_even, h_odd = 2 * jj, 2 * jj + 1
                wot[0:64, jj * E : (jj + 1) * E] = \
                    Wo[:, FL * g + 64 * h_even : FL * g + 64 * h_even + 64].T
                wot[64:128, jj * E : (jj + 1) * E] = \
                    Wo[:, FL * g + 64 * h_odd : FL * g + 64 * h_odd + 64].T
        else:
            wot = np.zeros((65, HL * E), f32)
            for h in range(HL):
                wot[1:65, E * h : E * h + E] = \
                    Wo[:, FL * g + 64 * h : FL * g + 64 * h + 64].T
        in_maps.append({
            "xqT": xT[b][0], "xkT": xT[b][1], "xvT": xT[b][2],
            "Wq": _pack_w(wq_c).astype(bf),
            "Wk": _pack_w(wk_c).astype(bf),
            "Wv": _pack_w(wv_pack).astype(bf),
            "bqk": bqk.astype(f32),
            "bvb": bv_bcast.astype(bf),
            "WoT": wot.astype(bf),
        })
    return in_maps


def assemble_output(results, bo):
    out = np.empty((B, S, E), np.float32)
    for b in range(B):
        acc = results[4 * b]["out"].astype(np.float32).copy()
        for g in range(1, 4):
            acc += results[4 * b + g]["out"]
        out[b] = acc + bo[None, :].astype(np.float32)
    return out


def kernel(query, key, value, Wq, bq, Wk, bk, Wv, bv, Wo, bo, **run_kwargs):
    nc = _get_nc()
    in_maps = make_in_maps(query, key, value, Wq, bq, Wk, bk, Wv, bv, Wo, bo)
    res = run_bass_kernel_spmd(nc, in_maps, core_ids=list(range(N_CORES)),
                               **run_kwargs)
    out = assemble_output(res.results, np.asarray(bo, np.float32))
    kernel.last_result = res
    return out
